# revision 2
# baseline (speedup 1.0000x reference)
"""MMD loss (RBF kernel, sigma=1) on 8 Trainium2 NeuronCores.

kernel(x, y): x, y float32 [20000, 64] -> float32 scalar
    kxx/nX^2 + kyy/nY^2 - 2*kxy/(nX*nY),  k** = sum_ij exp(-||a_i-b_j||^2/2)

Strategy
--------
exp(-(|a|^2+|b|^2-2ab)/2) = exp((a.b + s_b) + s_a), s_v = -|v|^2/2.
Per (row,col) tile: ONE fp16 matmul with K=66 (64 dims + hi/lo split of s_b)
-> PSUM, then ONE ScalarE Exp activation with per-partition bias s_a and
accum_out row-sums. Row blocks of 2500 are sharded across the 8 cores.
Symmetry of kxx/kyy: core c computes its row block against a circularly
rolled 12500-wide (5-block) column window; the double count for
distance-1..3 blocks is folded in by adding ln2 to s_b (exp(m+ln2)=2exp(m)).
Distance-4 blocks are computed by both paired cores once each (= the two
ordered pairs). kxy is rows x-block c vs all 20000 y columns.
Pad rows (2500->2560) are killed via bias=-60000 -> exp -> 0.
Host does the final (tiny) reduction of per-core [128, n_slots] partials.
"""

import os

import numpy as np

# problem dims (hardcoded per contract)
N = 20000
D = 64
CORES = 8
BLOCK = N // CORES  # 2500
TILE = 128
N_TILES = 20  # ceil(2500/128)
PAD_BLOCK = TILE * N_TILES  # 2560
KXX_SPAN = 5 * BLOCK  # 12500
K = D + 2  # 66 contraction rows
CHUNK = 2048  # ACT chunk (4 PSUM banks)
MM_N = 512  # matmul moving free dim (1 PSUM bank fp32)
LN2 = float(np.log(2.0))
KILL = -60000.0

_CACHE: dict = {}


def _chunks(total, chunk=CHUNK):
    out, pos = [], 0
    while pos < total:
        n = min(chunk, total - pos)
        out.append((pos, n))
        pos += n
    return out


# (cols_name, bias_name, rw_name, ncols, accumulator index)
_ITEMS = [
    ("colsxr", "biasx", "rwx", KXX_SPAN, 0),
    ("colsyr", "biasy", "rwy", KXX_SPAN, 1),
    ("colsyf", "biasx", "rwx", N, 2),
]


def _slot_meta():
    meta = []
    for _, _, _, ncols, acc in _ITEMS:
        for _r in range(N_TILES):
            for _c in _chunks(ncols):
                meta.append(acc)
    return meta


def _build_nc():
    import concourse.bacc as bacc
    import concourse.tile as tile
    from concourse import mybir

    n_slots = len(_slot_meta())

    nc = bacc.Bacc("TRN2", target_bir_lowering=False)
    f16 = mybir.dt.float16
    f32 = mybir.dt.float32

    dram = {
        "colsxr": nc.dram_tensor("colsxr", [K, KXX_SPAN], f16, kind="ExternalInput"),
        "colsyr": nc.dram_tensor("colsyr", [K, KXX_SPAN], f16, kind="ExternalInput"),
        "colsyf": nc.dram_tensor("colsyf", [K, N], f16, kind="ExternalInput"),
        "rwx": nc.dram_tensor("rwx", [K, PAD_BLOCK], f16, kind="ExternalInput"),
        "rwy": nc.dram_tensor("rwy", [K, PAD_BLOCK], f16, kind="ExternalInput"),
        "biasx": nc.dram_tensor("biasx", [TILE, N_TILES], f32, kind="ExternalInput"),
        "biasy": nc.dram_tensor("biasy", [TILE, N_TILES], f32, kind="ExternalInput"),
    }
    parts_d = nc.dram_tensor("parts", [TILE, n_slots], f32, kind="ExternalOutput")

    with tile.TileContext(nc) as tc:
        with (
            tc.tile_pool(name="sb", bufs=1) as sb,
            tc.tile_pool(name="ps", bufs=2, space="PSUM") as ps,
        ):
            colsxr = sb.tile([K, KXX_SPAN], f16)
            colsyr = sb.tile([K, KXX_SPAN], f16)
            colsyf = sb.tile([K, N], f16)
            rwx = sb.tile([K, PAD_BLOCK], f16)
            rwy = sb.tile([K, PAD_BLOCK], f16)
            biasx = sb.tile([TILE, N_TILES], f32)
            biasy = sb.tile([TILE, N_TILES], f32)
            parts = sb.tile([TILE, n_slots], f32)
            sbuf = {
                "colsxr": colsxr, "colsyr": colsyr, "colsyf": colsyf,
                "rwx": rwx, "rwy": rwy, "biasx": biasx, "biasy": biasy,
            }
            for name, t in sbuf.items():
                nc.sync.dma_start(out=t, in_=dram[name][:, :])

            slot = 0
            for cols_name, bias_name, rw_name, ncols, _acc in _ITEMS:
                cols, bias, rw = sbuf[cols_name], sbuf[bias_name], sbuf[rw_name]
                for r in range(N_TILES):
                    lhsT = rw[:, r * TILE : (r + 1) * TILE]
                    for c0, cn in _chunks(ncols):
                        pt = ps.tile([TILE, CHUNK], f32, tag="pt", name=f"pt{slot}")
                        for s0 in range(0, cn, MM_N):
                            sn = min(MM_N, cn - s0)
                            nc.tensor.matmul(
                                pt[:, s0 : s0 + sn],
                                lhsT,
                                cols[:, c0 + s0 : c0 + s0 + sn],
                                start=True,
                                stop=True,
                            )
                        nc.scalar.activation(
                            out=pt[:, :cn],
                            in_=pt[:, :cn],
                            func=mybir.ActivationFunctionType.Exp,
                            bias=bias[:, r : r + 1],
                            scale=1.0,
                            accum_out=parts[:, slot : slot + 1],
                        )
                        slot += 1
            nc.sync.dma_start(out=parts_d[:, :], in_=parts)
    nc.compile()
    return nc


def _prep_side(v):
    """v [N, D] fp32 -> (vh fp16 [N, D], s fp64 [N] = -|vh|^2/2)"""
    vh = v.astype(np.float16)
    s = -0.5 * np.sum(vh.astype(np.float64) ** 2, axis=1)
    return vh, s


def _cols_tensor(vh, s_adj):
    h = s_adj.astype(np.float16)
    l = (s_adj - h.astype(np.float64)).astype(np.float16)
    return np.ascontiguousarray(
        np.concatenate([vh.T, h[None], l[None]], axis=0)
    )


def _rw_tensor(vh_block):
    rw = np.zeros((K, PAD_BLOCK), dtype=np.float16)
    n = vh_block.shape[0]
    rw[:D, :n] = vh_block.T
    rw[D:, :n] = 1.0
    return rw


def _bias_tensor(s_block):
    b = np.full((TILE, N_TILES), KILL, dtype=np.float32)
    n = len(s_block)
    idx = np.arange(n)
    b[idx % TILE, idx // TILE] = s_block.astype(np.float32)
    return b


def _make_in_maps(x, y):
    xh, sx = _prep_side(x)
    yh, sy = _prep_side(y)
    colsyf = _cols_tensor(yh, sy)
    w2 = np.zeros(KXX_SPAN)
    w2[BLOCK : 4 * BLOCK] = LN2

    in_maps = []
    for c in range(CORES):
        order = (np.arange(KXX_SPAN) + BLOCK * c) % N
        blk = slice(BLOCK * c, BLOCK * (c + 1))
        in_maps.append(
            {
                "colsxr": _cols_tensor(xh[order], sx[order] + w2),
                "colsyr": _cols_tensor(yh[order], sy[order] + w2),
                "colsyf": colsyf,
                "rwx": _rw_tensor(xh[blk]),
                "rwy": _rw_tensor(yh[blk]),
                "biasx": _bias_tensor(sx[blk]),
                "biasy": _bias_tensor(sy[blk]),
            }
        )
    return in_maps


def kernel(x, y):
    from concourse.bass_utils import run_bass_kernel_spmd

    x = np.asarray(x, dtype=np.float32)
    y = np.asarray(y, dtype=np.float32)
    assert x.shape == (N, D) and y.shape == (N, D)

    if "nc" not in _CACHE:
        _CACHE["nc"] = _build_nc()
    nc = _CACHE["nc"]

    in_maps = _make_in_maps(x, y)
    trace = os.environ.get("MMD_TRACE", "0") == "1"
    try:
        br = run_bass_kernel_spmd(
            nc, in_maps, core_ids=list(range(CORES)), trace=trace
        )
    except Exception:
        if not trace:
            raise
        import traceback

        traceback.print_exc()
        print("trace run failed; retrying without trace")
        br = run_bass_kernel_spmd(
            nc, in_maps, core_ids=list(range(CORES)), trace=False
        )
    _CACHE["last_results"] = br

    meta = np.array(_slot_meta())
    tot = np.zeros(3, dtype=np.float64)
    for core_res in br.results:
        sums = core_res["parts"].astype(np.float64).sum(axis=0)
        for acc in range(3):
            tot[acc] += float(sums[meta == acc].sum())
    val = tot[0] / (N * N) + tot[1] / (N * N) - 2.0 * tot[2] / (N * N)
    return np.array(val, dtype=np.float32)


# revision 5
# speedup vs baseline: 1.0150x; 1.0150x over previous
"""MMD loss (RBF kernel, sigma=1) on 8 Trainium2 NeuronCores.

kernel(x, y): x, y float32 [20000, 64] -> float32 scalar
    kxx/nX^2 + kyy/nY^2 - 2*kxy/(nX*nY),  k** = sum_ij exp(-||a_i-b_j||^2/2)

Strategy
--------
exp(-(|a|^2+|b|^2-2ab)/2) = exp((a.b + s_b) + s_a), s_v = -|v|^2/2.
Per (row,col) tile: ONE fp16 matmul with K=66 (64 dims + hi/lo split of s_b)
-> PSUM, then ONE ScalarE Exp activation with per-partition bias s_a and
accum_out row-sums. Row blocks of 2500 are sharded across the 8 cores.
Symmetry of kxx/kyy: core c computes its row block against a circularly
rolled 12500-wide (5-block) column window; the double count for
distance-1..3 blocks is folded in by adding ln2 to s_b (exp(m+ln2)=2exp(m)).
Distance-4 blocks are computed by both paired cores once each (= the two
ordered pairs). kxy is rows x-block c vs all 20000 y columns.
Pad rows (2500->2560) are killed via bias=-60000 -> exp -> 0.
Host does the final (tiny) reduction of per-core [128, n_slots] partials.
"""

import os

import numpy as np

# problem dims (hardcoded per contract)
N = 20000
D = 64
CORES = 8
BLOCK = N // CORES  # 2500
TILE = 128
N_TILES = 20  # ceil(2500/128)
PAD_BLOCK = TILE * N_TILES  # 2560
KXX_SPAN = 5 * BLOCK  # 12500
K = D + 2  # 66 contraction rows
CHUNK = 2048  # ACT chunk (4 PSUM banks)
MM_N = 512  # matmul moving free dim (1 PSUM bank fp32)
LN2 = float(np.log(2.0))
KILL = -60000.0

_CACHE: dict = {}


def _chunks(total, chunk=CHUNK):
    out, pos = [], 0
    while pos < total:
        n = min(chunk, total - pos)
        out.append((pos, n))
        pos += n
    return out


# (cols_name, bias_name, rw_name, ncols, accumulator index)
_ITEMS = [
    ("colsxr", "biasx", "rwx", KXX_SPAN, 0),
    ("colsyr", "biasy", "rwy", KXX_SPAN, 1),
    ("colsyf", "biasx", "rwx", N, 2),
]


def _slot_meta():
    meta = []
    for _, _, _, ncols, acc in _ITEMS:
        for _r in range(N_TILES):
            for _c in _chunks(ncols):
                meta.append(acc)
    return meta


def _build_nc():
    import concourse.bacc as bacc
    import concourse.tile as tile
    from concourse import mybir

    n_slots = len(_slot_meta())

    nc = bacc.Bacc("TRN2", target_bir_lowering=False)
    f16 = mybir.dt.float16
    f32 = mybir.dt.float32

    dram = {
        "colsxr": nc.dram_tensor("colsxr", [K, KXX_SPAN], f16, kind="ExternalInput"),
        "colsyr": nc.dram_tensor("colsyr", [K, KXX_SPAN], f16, kind="ExternalInput"),
        "colsyf": nc.dram_tensor("colsyf", [K, N], f16, kind="ExternalInput"),
        "rwx": nc.dram_tensor("rwx", [K, PAD_BLOCK], f16, kind="ExternalInput"),
        "rwy": nc.dram_tensor("rwy", [K, PAD_BLOCK], f16, kind="ExternalInput"),
        "biasx": nc.dram_tensor("biasx", [TILE, N_TILES], f32, kind="ExternalInput"),
        "biasy": nc.dram_tensor("biasy", [TILE, N_TILES], f32, kind="ExternalInput"),
    }
    parts_d = nc.dram_tensor("parts", [TILE, n_slots], f32, kind="ExternalOutput")

    with tile.TileContext(nc) as tc:
        with (
            tc.tile_pool(name="sb", bufs=1) as sb,
            tc.tile_pool(name="ps", bufs=2, space="PSUM") as ps,
        ):
            colsxr = sb.tile([K, KXX_SPAN], f16)
            colsyr = sb.tile([K, KXX_SPAN], f16)
            colsyf = sb.tile([K, N], f16)
            rwx = sb.tile([K, PAD_BLOCK], f16)
            rwy = sb.tile([K, PAD_BLOCK], f16)
            biasx = sb.tile([TILE, N_TILES], f32)
            biasy = sb.tile([TILE, N_TILES], f32)
            parts = sb.tile([TILE, n_slots], f32)
            sbuf = {
                "colsxr": colsxr, "colsyr": colsyr, "colsyf": colsyf,
                "rwx": rwx, "rwy": rwy, "biasx": biasx, "biasy": biasy,
            }
            # Small tensors first (warmup matmuls depend on rwx), then the
            # big cols tensors split into pieces spread over several DGE
            # queues so the first compute chunk starts ASAP.
            for name in ("rwx", "rwy", "biasx", "biasy"):
                nc.sync.dma_start(out=sbuf[name], in_=dram[name][:, :])
            dma_engines = [nc.sync, nc.gpsimd]
            ei = 0
            for name in ("colsxr", "colsyr", "colsyf"):
                t = sbuf[name]
                total = t.shape[-1]
                npieces = 4
                step = (total + npieces - 1) // npieces
                for p0 in range(0, total, step):
                    p1 = min(p0 + step, total)
                    dma_engines[ei % len(dma_engines)].dma_start(
                        out=t[:, p0:p1], in_=dram[name][:, p0:p1]
                    )
                    ei += 1

            # PE warmup: ~24 dense matmuls on the (tiny, early) rwx tile so
            # the HAM clock-gate opens (1.2 -> 2.4 GHz) before real work and
            # stays open (real-work PE gaps are << the ~3.4us MID window).
            warm_pt = ps.tile([TILE, CHUNK], f32, tag="pt", name="warm_pt")
            for w in range(24):
                s0 = (w % 4) * MM_N
                nc.tensor.matmul(
                    warm_pt[:, s0 : s0 + MM_N],
                    rwx[:, :TILE],
                    rwx[:, s0 : s0 + MM_N],
                    start=True,
                    stop=True,
                )
            warm_junk = sb.tile([TILE, 1], f32)
            nc.scalar.activation(
                out=warm_junk,
                in_=warm_pt[:, :1],
                func=mybir.ActivationFunctionType.Exp,
                bias=biasx[:, 0:1],
                scale=0.0,
            )

            slot = 0
            for cols_name, bias_name, rw_name, ncols, _acc in _ITEMS:
                cols, bias, rw = sbuf[cols_name], sbuf[bias_name], sbuf[rw_name]
                for r in range(N_TILES):
                    lhsT = rw[:, r * TILE : (r + 1) * TILE]
                    for c0, cn in _chunks(ncols):
                        pt = ps.tile([TILE, CHUNK], f32, tag="pt", name=f"pt{slot}")
                        for s0 in range(0, cn, MM_N):
                            sn = min(MM_N, cn - s0)
                            nc.tensor.matmul(
                                pt[:, s0 : s0 + sn],
                                lhsT,
                                cols[:, c0 + s0 : c0 + s0 + sn],
                                start=True,
                                stop=True,
                            )
                        nc.scalar.activation(
                            out=pt[:, :cn],
                            in_=pt[:, :cn],
                            func=mybir.ActivationFunctionType.Exp,
                            bias=bias[:, r : r + 1],
                            scale=1.0,
                            accum_out=parts[:, slot : slot + 1],
                        )
                        slot += 1
            nc.sync.dma_start(out=parts_d[:, :], in_=parts)
    nc.compile()
    return nc


def _prep_side(v):
    """v [N, D] fp32 -> (vh fp16 [N, D], s fp64 [N] = -|vh|^2/2)"""
    vh = v.astype(np.float16)
    s = -0.5 * np.sum(vh.astype(np.float64) ** 2, axis=1)
    return vh, s


def _cols_tensor(vh, s_adj):
    h = s_adj.astype(np.float16)
    l = (s_adj - h.astype(np.float64)).astype(np.float16)
    return np.ascontiguousarray(
        np.concatenate([vh.T, h[None], l[None]], axis=0)
    )


def _rw_tensor(vh_block):
    rw = np.zeros((K, PAD_BLOCK), dtype=np.float16)
    n = vh_block.shape[0]
    rw[:D, :n] = vh_block.T
    rw[D:, :n] = 1.0
    return rw


def _bias_tensor(s_block):
    b = np.full((TILE, N_TILES), KILL, dtype=np.float32)
    n = len(s_block)
    idx = np.arange(n)
    b[idx % TILE, idx // TILE] = s_block.astype(np.float32)
    return b


def _make_in_maps(x, y):
    xh, sx = _prep_side(x)
    yh, sy = _prep_side(y)
    colsyf = _cols_tensor(yh, sy)
    w2 = np.zeros(KXX_SPAN)
    w2[BLOCK : 4 * BLOCK] = LN2

    in_maps = []
    for c in range(CORES):
        order = (np.arange(KXX_SPAN) + BLOCK * c) % N
        blk = slice(BLOCK * c, BLOCK * (c + 1))
        in_maps.append(
            {
                "colsxr": _cols_tensor(xh[order], sx[order] + w2),
                "colsyr": _cols_tensor(yh[order], sy[order] + w2),
                "colsyf": colsyf,
                "rwx": _rw_tensor(xh[blk]),
                "rwy": _rw_tensor(yh[blk]),
                "biasx": _bias_tensor(sx[blk]),
                "biasy": _bias_tensor(sy[blk]),
            }
        )
    return in_maps


def kernel(x, y):
    from concourse.bass_utils import run_bass_kernel_spmd

    x = np.asarray(x, dtype=np.float32)
    y = np.asarray(y, dtype=np.float32)
    assert x.shape == (N, D) and y.shape == (N, D)

    if "nc" not in _CACHE:
        _CACHE["nc"] = _build_nc()
    nc = _CACHE["nc"]

    in_maps = _make_in_maps(x, y)
    trace = os.environ.get("MMD_TRACE", "0") == "1"
    try:
        br = run_bass_kernel_spmd(
            nc, in_maps, core_ids=list(range(CORES)), trace=trace
        )
    except Exception:
        if not trace:
            raise
        import traceback

        traceback.print_exc()
        print("trace run failed; retrying without trace")
        br = run_bass_kernel_spmd(
            nc, in_maps, core_ids=list(range(CORES)), trace=False
        )
    _CACHE["last_results"] = br

    meta = np.array(_slot_meta())
    tot = np.zeros(3, dtype=np.float64)
    for core_res in br.results:
        sums = core_res["parts"].astype(np.float64).sum(axis=0)
        for acc in range(3):
            tot[acc] += float(sums[meta == acc].sum())
    val = tot[0] / (N * N) + tot[1] / (N * N) - 2.0 * tot[2] / (N * N)
    return np.array(val, dtype=np.float32)


# revision 7
# speedup vs baseline: 1.0907x; 1.0746x over previous
"""MMD loss (RBF kernel, sigma=1) on 8 Trainium2 NeuronCores.

kernel(x, y): x, y float32 [20000, 64] -> float32 scalar
    kxx/nX^2 + kyy/nY^2 - 2*kxy/(nX*nY),  k** = sum_ij exp(-||a_i-b_j||^2/2)

Strategy
--------
exp(-(|a|^2+|b|^2-2ab)/2) = exp((a.b + s_b) + s_a), s_v = -|v|^2/2.
Per (row,col) tile: ONE fp16 matmul with K=66 (64 dims + hi/lo split of s_b)
-> PSUM, then ONE ScalarE Exp activation with per-partition bias s_a and
accum_out row-sums. Row blocks of 2500 are sharded across the 8 cores.
Symmetry of kxx/kyy: core c computes its row block against a circularly
rolled 12500-wide (5-block) column window; the double count for
distance-1..3 blocks is folded in by adding ln2 to s_b (exp(m+ln2)=2exp(m)).
Distance-4 blocks are computed by both paired cores once each (= the two
ordered pairs). kxy is rows x-block c vs all 20000 y columns.
Pad rows (2500->2560) are killed via bias=-60000 -> exp -> 0.
Host does the final (tiny) reduction of per-core [128, n_slots] partials.
"""

import os

import numpy as np

# problem dims (hardcoded per contract)
N = 20000
D = 64
CORES = 8
BLOCK = N // CORES  # 2500
TILE = 128
N_TILES = 20  # ceil(2500/128)
PAD_BLOCK = TILE * N_TILES  # 2560
KXX_SPAN = 5 * BLOCK  # 12500
K = D + 2  # 66 contraction rows
CHUNK = 2048  # ACT chunk (4 PSUM banks)
MM_N = 512  # matmul moving free dim (1 PSUM bank fp32)
LN2 = float(np.log(2.0))
KILL = -60000.0

_CACHE: dict = {}


def _chunks(total, chunk=CHUNK):
    """Split `total` cols into equal-width chunks (each <= chunk).

    Uniform widths keep the ACT time per chunk >= the PE time per chunk in
    the 2-deep PSUM pipeline; a short tail chunk would let ACT run dry and
    cost a ~1.5us bubble per row-tile."""
    n = -(-total // chunk)  # ceil
    base, rem = divmod(total, n)
    out, pos = [], 0
    for i in range(n):
        w = base + (1 if i < rem else 0)
        out.append((pos, w))
        pos += w
    return out


# (cols_name, bias_name, rw_name, ncols, accumulator index)
_ITEMS = [
    ("colsxr", "biasx", "rwx", KXX_SPAN, 0),
    ("colsyr", "biasy", "rwy", KXX_SPAN, 1),
    ("colsyf", "biasx", "rwx", N, 2),
]


def _slot_meta():
    meta = []
    for _, _, _, ncols, acc in _ITEMS:
        for _r in range(N_TILES):
            for _c in _chunks(ncols):
                meta.append(acc)
    return meta


def _build_nc():
    import concourse.bacc as bacc
    import concourse.tile as tile
    from concourse import mybir

    n_slots = len(_slot_meta())

    nc = bacc.Bacc("TRN2", target_bir_lowering=False)
    f16 = mybir.dt.float16
    f32 = mybir.dt.float32

    dram = {
        "colsxr": nc.dram_tensor("colsxr", [K, KXX_SPAN], f16, kind="ExternalInput"),
        "colsyr": nc.dram_tensor("colsyr", [K, KXX_SPAN], f16, kind="ExternalInput"),
        "colsyf": nc.dram_tensor("colsyf", [K, N], f16, kind="ExternalInput"),
        "rwx": nc.dram_tensor("rwx", [K, PAD_BLOCK], f16, kind="ExternalInput"),
        "rwy": nc.dram_tensor("rwy", [K, PAD_BLOCK], f16, kind="ExternalInput"),
        "biasx": nc.dram_tensor("biasx", [TILE, N_TILES], f32, kind="ExternalInput"),
        "biasy": nc.dram_tensor("biasy", [TILE, N_TILES], f32, kind="ExternalInput"),
    }
    parts_d = nc.dram_tensor("parts", [TILE, n_slots], f32, kind="ExternalOutput")

    with tile.TileContext(nc) as tc:
        with (
            tc.tile_pool(name="sb", bufs=1) as sb,
            tc.tile_pool(name="ps", bufs=2, space="PSUM") as ps,
        ):
            colsxr = sb.tile([K, KXX_SPAN], f16)
            colsyr = sb.tile([K, KXX_SPAN], f16)
            colsyf = sb.tile([K, N], f16)
            rwx = sb.tile([K, PAD_BLOCK], f16)
            rwy = sb.tile([K, PAD_BLOCK], f16)
            biasx = sb.tile([TILE, N_TILES], f32)
            biasy = sb.tile([TILE, N_TILES], f32)
            parts = sb.tile([TILE, n_slots], f32)
            sbuf = {
                "colsxr": colsxr, "colsyr": colsyr, "colsyf": colsyf,
                "rwx": rwx, "rwy": rwy, "biasx": biasx, "biasy": biasy,
            }
            # Small tensors first, then the big cols tensors split into
            # pieces spread over the two DGE queues; the leading colsxr
            # piece is small so the first compute chunk starts ASAP.
            for name in ("rwx", "rwy", "biasx", "biasy"):
                nc.sync.dma_start(out=sbuf[name], in_=dram[name][:, :])
            dma_engines = [nc.sync, nc.gpsimd]
            ei = 0
            for name in ("colsxr", "colsyr", "colsyf"):
                t = sbuf[name]
                total = t.shape[-1]
                pieces = [2048] if name == "colsxr" else []
                left = total - sum(pieces)
                npieces = 4
                step = (left + npieces - 1) // npieces
                while left > 0:
                    w = min(step, left)
                    pieces.append(w)
                    left -= w
                p0 = 0
                for w in pieces:
                    dma_engines[ei % len(dma_engines)].dma_start(
                        out=t[:, p0 : p0 + w], in_=dram[name][:, p0 : p0 + w]
                    )
                    p0 += w
                    ei += 1

            slot = 0
            for cols_name, bias_name, rw_name, ncols, _acc in _ITEMS:
                cols, bias, rw = sbuf[cols_name], sbuf[bias_name], sbuf[rw_name]
                for r in range(N_TILES):
                    lhsT = rw[:, r * TILE : (r + 1) * TILE]
                    for c0, cn in _chunks(ncols):
                        pt = ps.tile([TILE, CHUNK], f32, tag="pt", name=f"pt{slot}")
                        for s0 in range(0, cn, MM_N):
                            sn = min(MM_N, cn - s0)
                            nc.tensor.matmul(
                                pt[:, s0 : s0 + sn],
                                lhsT,
                                cols[:, c0 + s0 : c0 + s0 + sn],
                                start=True,
                                stop=True,
                            )
                        nc.scalar.activation(
                            out=pt[:, :cn],
                            in_=pt[:, :cn],
                            func=mybir.ActivationFunctionType.Exp,
                            bias=bias[:, r : r + 1],
                            scale=1.0,
                            accum_out=parts[:, slot : slot + 1],
                        )
                        slot += 1
            nc.sync.dma_start(out=parts_d[:, :], in_=parts)
    nc.compile()
    return nc


def _prep_side(v):
    """v [N, D] fp32 -> (vh fp16 [N, D], s fp64 [N] = -|vh|^2/2)"""
    vh = v.astype(np.float16)
    s = -0.5 * np.sum(vh.astype(np.float64) ** 2, axis=1)
    return vh, s


def _cols_tensor(vh, s_adj):
    h = s_adj.astype(np.float16)
    l = (s_adj - h.astype(np.float64)).astype(np.float16)
    return np.ascontiguousarray(
        np.concatenate([vh.T, h[None], l[None]], axis=0)
    )


def _rw_tensor(vh_block):
    rw = np.zeros((K, PAD_BLOCK), dtype=np.float16)
    n = vh_block.shape[0]
    rw[:D, :n] = vh_block.T
    rw[D:, :n] = 1.0
    return rw


def _bias_tensor(s_block):
    b = np.full((TILE, N_TILES), KILL, dtype=np.float32)
    n = len(s_block)
    idx = np.arange(n)
    b[idx % TILE, idx // TILE] = s_block.astype(np.float32)
    return b


def _make_in_maps(x, y):
    xh, sx = _prep_side(x)
    yh, sy = _prep_side(y)
    colsyf = _cols_tensor(yh, sy)
    w2 = np.zeros(KXX_SPAN)
    w2[BLOCK : 4 * BLOCK] = LN2

    in_maps = []
    for c in range(CORES):
        order = (np.arange(KXX_SPAN) + BLOCK * c) % N
        blk = slice(BLOCK * c, BLOCK * (c + 1))
        in_maps.append(
            {
                "colsxr": _cols_tensor(xh[order], sx[order] + w2),
                "colsyr": _cols_tensor(yh[order], sy[order] + w2),
                "colsyf": colsyf,
                "rwx": _rw_tensor(xh[blk]),
                "rwy": _rw_tensor(yh[blk]),
                "biasx": _bias_tensor(sx[blk]),
                "biasy": _bias_tensor(sy[blk]),
            }
        )
    return in_maps


def kernel(x, y):
    from concourse.bass_utils import run_bass_kernel_spmd

    x = np.asarray(x, dtype=np.float32)
    y = np.asarray(y, dtype=np.float32)
    assert x.shape == (N, D) and y.shape == (N, D)

    if "nc" not in _CACHE:
        _CACHE["nc"] = _build_nc()
    nc = _CACHE["nc"]

    in_maps = _make_in_maps(x, y)
    trace = os.environ.get("MMD_TRACE", "0") == "1"
    try:
        br = run_bass_kernel_spmd(
            nc, in_maps, core_ids=list(range(CORES)), trace=trace
        )
    except Exception:
        if not trace:
            raise
        import traceback

        traceback.print_exc()
        print("trace run failed; retrying without trace")
        br = run_bass_kernel_spmd(
            nc, in_maps, core_ids=list(range(CORES)), trace=False
        )
    _CACHE["last_results"] = br

    meta = np.array(_slot_meta())
    tot = np.zeros(3, dtype=np.float64)
    for core_res in br.results:
        sums = core_res["parts"].astype(np.float64).sum(axis=0)
        for acc in range(3):
            tot[acc] += float(sums[meta == acc].sum())
    val = tot[0] / (N * N) + tot[1] / (N * N) - 2.0 * tot[2] / (N * N)
    return np.array(val, dtype=np.float32)


# revision 8
# speedup vs baseline: 1.1356x; 1.0412x over previous
"""MMD loss (RBF kernel, sigma=1) on 8 Trainium2 NeuronCores.

kernel(x, y): x, y float32 [20000, 64] -> float32 scalar
    kxx/nX^2 + kyy/nY^2 - 2*kxy/(nX*nY),  k** = sum_ij exp(-||a_i-b_j||^2/2)

Strategy
--------
exp(-(|a|^2+|b|^2-2ab)/2) = exp(a.b + s_a + s_b), s_v = -|v|^2/2.
The whole exponent is produced by ONE fp16 matmul with K=69:
row vector [a (64); ha; la; 1; 1] x col vector [b (64); 1; 1; gb; gl],
where ha+la is an fp16 hi/lo split of s_a and gb+gl of s_b (+ln2 weight).
Then ONE ScalarE Exp activation (bias 0) with accum_out row-sums per
PSUM chunk. ScalarE is the bottleneck engine (1 elem/lane/cycle); the
layout keeps it ~96% busy.

Sharding: row blocks of 2500 across 8 cores (SPMD, identical program).
kxx/kyy symmetry, exactly:
  - core c's column window = x-cols rolled by 2500c, width 12500
    (its own block + the next 4 blocks);
  - within-window weights: cols [0:10000) carry +ln2 in g (doubles the
    term, exp(m+ln2)=2exp(m)) and each row-tile r only covers cols
    [128(r+1), 12500) -> strictly-upper cross-tile pairs counted once
    with weight 2; distance-4 cols [10000:12500) carry no ln2 and are
    computed by both paired cores (= the two ordered block pairs).
  - the in-tile 128x128 diagonal squares (incl. the true diagonal) are
    computed once, weight 1, by a small "coda" of 40 squares that runs
    during the input-DMA ramp.
Pad rows/cols are killed inside the exponent (-30000 components -> exp=0).
Host does the final (tiny) reduction of per-core [128, n_slots] partials.
"""

import os

import numpy as np

# problem dims (hardcoded per contract)
N = 20000
D = 64
CORES = 8
BLOCK = N // CORES  # 2500
TILE = 128
N_TILES = 20  # ceil(2500/128)
PAD_BLOCK = TILE * N_TILES  # 2560
KXX_SPAN = 5 * BLOCK  # 12500
K = D + 5  # 69 contraction rows
CHUNK = 2048  # ACT chunk (4 PSUM banks)
MM_N = 512  # matmul moving free dim (1 PSUM bank fp32)
LN2 = float(np.log(2.0))
KILL = np.float16(-30000.0)  # x2 slots -> -60000 -> exp underflows to 0

_CACHE: dict = {}


def _eq_chunks(total, chunk=CHUNK):
    """Equal-width chunks (each <= chunk). Uniform widths keep ACT-per-chunk
    >= PE-per-chunk in the 2-deep PSUM pipeline (no ACT starvation)."""
    if total <= 0:
        return []
    n = -(-total // chunk)
    base, rem = divmod(total, n)
    out, pos = [], 0
    for i in range(n):
        w = base + (1 if i < rem else 0)
        out.append((pos, w))
        pos += w
    return out


# (cols_name, rw_name, ncols, accumulator index, triangle?)
_ITEMS = [
    ("colsxr", "rwx", KXX_SPAN, 0, True),
    ("colsyr", "rwy", KXX_SPAN, 1, True),
    ("colsyf", "rwx", N, 2, False),
]


def _slot_meta():
    meta = [0, 0, 1, 1]  # coda: two x-square chunks, two y-square chunks
    for _, _, ncols, acc, tri in _ITEMS:
        for r in range(N_TILES):
            base = TILE * (r + 1) if tri else 0
            for _c in _eq_chunks(ncols - base):
                meta.append(acc)
    return meta


def _build_nc():
    import concourse.bacc as bacc
    import concourse.tile as tile
    from concourse import mybir

    n_slots = len(_slot_meta())

    nc = bacc.Bacc("TRN2", target_bir_lowering=False)
    f16 = mybir.dt.float16
    f32 = mybir.dt.float32
    EXP = mybir.ActivationFunctionType.Exp

    dram = {
        "colsxr": nc.dram_tensor("colsxr", [K, KXX_SPAN], f16, kind="ExternalInput"),
        "colsyr": nc.dram_tensor("colsyr", [K, KXX_SPAN], f16, kind="ExternalInput"),
        "colsyf": nc.dram_tensor("colsyf", [K, N], f16, kind="ExternalInput"),
        "colsqx": nc.dram_tensor("colsqx", [K, PAD_BLOCK], f16, kind="ExternalInput"),
        "colsqy": nc.dram_tensor("colsqy", [K, PAD_BLOCK], f16, kind="ExternalInput"),
        "rwx": nc.dram_tensor("rwx", [K, PAD_BLOCK], f16, kind="ExternalInput"),
        "rwy": nc.dram_tensor("rwy", [K, PAD_BLOCK], f16, kind="ExternalInput"),
    }
    parts_d = nc.dram_tensor("parts", [TILE, n_slots], f32, kind="ExternalOutput")

    with tile.TileContext(nc) as tc:
        with (
            tc.tile_pool(name="sb", bufs=1) as sb,
            tc.tile_pool(name="ps", bufs=2, space="PSUM") as ps,
        ):
            colsxr = sb.tile([K, KXX_SPAN], f16)
            colsyr = sb.tile([K, KXX_SPAN], f16)
            colsyf = sb.tile([K, N], f16)
            colsqx = sb.tile([K, PAD_BLOCK], f16)
            colsqy = sb.tile([K, PAD_BLOCK], f16)
            rwx = sb.tile([K, PAD_BLOCK], f16)
            rwy = sb.tile([K, PAD_BLOCK], f16)
            parts = sb.tile([TILE, n_slots], f32)
            zeros = sb.tile([TILE, 1], f32)
            nc.vector.memset(zeros, 0.0)
            sbuf = {
                "colsxr": colsxr, "colsyr": colsyr, "colsyf": colsyf,
                "colsqx": colsqx, "colsqy": colsqy, "rwx": rwx, "rwy": rwy,
            }
            # Coda inputs + first-needed cols piece first; big tensors in
            # pieces over the two DGE queues.
            for name in ("colsqx", "colsqy", "rwx", "rwy"):
                nc.sync.dma_start(out=sbuf[name], in_=dram[name][:, :])
            dma_engines = [nc.sync, nc.gpsimd]
            ei = 0
            for name in ("colsxr", "colsyr", "colsyf"):
                t = sbuf[name]
                total = t.shape[-1]
                pieces = [2048] if name == "colsxr" else []
                left = total - sum(pieces)
                step = -(-left // 4)
                while left > 0:
                    w = min(step, left)
                    pieces.append(w)
                    left -= w
                p0 = 0
                for w in pieces:
                    dma_engines[ei % len(dma_engines)].dma_start(
                        out=t[:, p0 : p0 + w], in_=dram[name][:, p0 : p0 + w]
                    )
                    p0 += w
                    ei += 1

            slot = 0

            def act_chunk(pt, cn, slot):
                nc.scalar.activation(
                    out=pt[:, :cn],
                    in_=pt[:, :cn],
                    func=EXP,
                    bias=zeros[:, 0:1],
                    scale=1.0,
                    accum_out=parts[:, slot : slot + 1],
                )

            # --- coda: 40 in-tile diagonal squares, 10 per PSUM chunk ---
            for rw, colsq in ((rwx, colsqx), (rwy, colsqy)):
                for half in range(2):
                    pt = ps.tile([TILE, CHUNK], f32, tag="pt", name=f"ptc{slot}")
                    for k in range(10):
                        r = 10 * half + k
                        sl = slice(TILE * r, TILE * (r + 1))
                        nc.tensor.matmul(
                            pt[:, TILE * k : TILE * (k + 1)],
                            rw[:, sl],
                            colsq[:, sl],
                            start=True,
                            stop=True,
                        )
                    act_chunk(pt, TILE * 10, slot)
                    slot += 1

            # --- main items ---
            for cols_name, rw_name, ncols, _acc, tri in _ITEMS:
                cols, rw = sbuf[cols_name], sbuf[rw_name]
                for r in range(N_TILES):
                    lhsT = rw[:, r * TILE : (r + 1) * TILE]
                    base = TILE * (r + 1) if tri else 0
                    for c0r, cn in _eq_chunks(ncols - base):
                        c0 = base + c0r
                        pt = ps.tile([TILE, CHUNK], f32, tag="pt", name=f"pt{slot}")
                        for s0 in range(0, cn, MM_N):
                            sn = min(MM_N, cn - s0)
                            nc.tensor.matmul(
                                pt[:, s0 : s0 + sn],
                                lhsT,
                                cols[:, c0 + s0 : c0 + s0 + sn],
                                start=True,
                                stop=True,
                            )
                        act_chunk(pt, cn, slot)
                        slot += 1
            nc.sync.dma_start(out=parts_d[:, :], in_=parts)
    nc.compile()
    return nc


def _prep_side(v):
    """v [N, D] fp32 -> (vh fp16 [N, D], s fp64 [N] = -|vh|^2/2)"""
    vh = v.astype(np.float16)
    s = -0.5 * np.sum(vh.astype(np.float64) ** 2, axis=1)
    return vh, s


def _hilo(s):
    h = s.astype(np.float16)
    l = (s - h.astype(np.float64)).astype(np.float16)
    return h, l


def _cols_tensor(vh, g):
    """[K, n] fp16 column tensor: [b; 1; 1; gh; gl]."""
    n = vh.shape[0]
    out = np.zeros((K, n), dtype=np.float16)
    out[:D] = vh.T
    out[D] = 1.0
    out[D + 1] = 1.0
    out[D + 2], out[D + 3] = _hilo(g)
    return np.ascontiguousarray(out)


def _rw_tensor(vh_block, s_block):
    """[K, PAD_BLOCK] fp16 row tensor: [a; ha; la; 1; 1]; pad rows killed."""
    n = vh_block.shape[0]
    rw = np.zeros((K, PAD_BLOCK), dtype=np.float16)
    rw[:D, :n] = vh_block.T
    rw[D, :n], rw[D + 1, :n] = _hilo(s_block)
    rw[D, n:] = KILL  # pad rows: ha * 1 = -30000 -> exp -> 0
    rw[D + 2, :n] = 1.0
    rw[D + 3, :n] = 1.0
    return rw


def _colsq_tensor(vh_block, s_block):
    """Coda columns: own block padded to PAD_BLOCK, pad cols killed."""
    n = vh_block.shape[0]
    vh_pad = np.zeros((PAD_BLOCK, D), dtype=np.float16)
    vh_pad[:n] = vh_block
    g = np.full(PAD_BLOCK, float(KILL), dtype=np.float64)
    g[:n] = s_block
    return _cols_tensor(vh_pad, g)


def _make_in_maps(x, y):
    xh, sx = _prep_side(x)
    yh, sy = _prep_side(y)
    colsyf = _cols_tensor(yh, sy)
    w2 = np.zeros(KXX_SPAN)
    w2[: 4 * BLOCK] = LN2  # diag-block uppers + distance 1..3: doubled

    in_maps = []
    for c in range(CORES):
        order = (np.arange(KXX_SPAN) + BLOCK * c) % N
        blk = slice(BLOCK * c, BLOCK * (c + 1))
        in_maps.append(
            {
                "colsxr": _cols_tensor(xh[order], sx[order] + w2),
                "colsyr": _cols_tensor(yh[order], sy[order] + w2),
                "colsyf": colsyf,
                "colsqx": _colsq_tensor(xh[blk], sx[blk]),
                "colsqy": _colsq_tensor(yh[blk], sy[blk]),
                "rwx": _rw_tensor(xh[blk], sx[blk]),
                "rwy": _rw_tensor(yh[blk], sy[blk]),
            }
        )
    return in_maps


def kernel(x, y):
    from concourse.bass_utils import run_bass_kernel_spmd

    x = np.asarray(x, dtype=np.float32)
    y = np.asarray(y, dtype=np.float32)
    assert x.shape == (N, D) and y.shape == (N, D)

    if "nc" not in _CACHE:
        _CACHE["nc"] = _build_nc()
    nc = _CACHE["nc"]

    in_maps = _make_in_maps(x, y)
    trace = os.environ.get("MMD_TRACE", "0") == "1"
    try:
        br = run_bass_kernel_spmd(
            nc, in_maps, core_ids=list(range(CORES)), trace=trace
        )
    except Exception:
        if not trace:
            raise
        import traceback

        traceback.print_exc()
        print("trace run failed; retrying without trace")
        br = run_bass_kernel_spmd(
            nc, in_maps, core_ids=list(range(CORES)), trace=False
        )
    _CACHE["last_results"] = br

    meta = np.array(_slot_meta())
    tot = np.zeros(3, dtype=np.float64)
    for core_res in br.results:
        sums = core_res["parts"].astype(np.float64).sum(axis=0)
        for acc in range(3):
            tot[acc] += float(sums[meta == acc].sum())
    val = tot[0] / (N * N) + tot[1] / (N * N) - 2.0 * tot[2] / (N * N)
    return np.array(val, dtype=np.float32)
